# revision 13
# baseline (speedup 1.0000x reference)
"""Trainium2 Bass kernel for nn_AdaptivePoolingClassifier (8 NeuronCores).

Math: the reference MLP is linear up to its single ReLU, so W1..W3 fold
into one 128x128 matrix on the host:
    h   = relu(x @ Wc^T + bc)       Wc = W3 W2 W1 ; bc = W3(W2 b1+b2)+b3
    p   = h @ W4^T + b4
    out = sum_n p * softmax(alpha*p, axis=1)

Device computes (rows sharded 8 ways):
    pt  = h @ (diag(alpha) W4)^T        # = alpha*(p - b4), [rows, 5]
    den_partial = sum_rows exp(pt) ; num_partial = sum_rows pt*exp(pt)
Host finishes: out_o = num_o/(alpha_o*den_o) + b4_o.

Dtypes: layer 1 runs all-fp8 (x and Wc — x noise averages out over the
200k-row softmax pool, Wc costs ~5e-3 systematic, well under the gate);
layer 4 runs all-bf16 (h, W4a — fp8 W4a was the error killer at 3e-2,
and mixed-dtype matmuls lower to a slow PE path so operands must match).

Layout: x is transposed on the host to [128(feat), rows] so features sit
on SBUF partitions for the folded matmul; layer-4 uses h chunks as the
matmul *stationary* operand so pt lands rows-on-partitions, making the
pooling full-width [128, 5*slots] ops instead of lane-starved [5, n]
ones. The whole fp8 x shard (25KB/partition) stays resident in SBUF via
5 ascending-size DMAs. ReLU is split between ACT and DVE by column range
(chunk-aligned). A few warmup matmuls on the constants keep the PE
p-state ramp going while the first x block streams in.
"""

import numpy as np
import ml_dtypes

from concourse import bacc, mybir, tile
from concourse.bass_utils import run_bass_kernel_spmd

N_CORES = 8
N_ROWS = 200000
F = 128
OUT = 5

ROWS_PAD = 200704            # 8 * 25088
RPC = ROWS_PAD // N_CORES    # rows per core = 25088 = 512 + 24*1024
T0 = 512                     # first tile
TILE = 1024                  # steady-state compute tile (2 PSUM banks)
N_TILES = (RPC - T0) // TILE  # 24
# uniform DMA group size after the wct and T0 transfers: fine-grained
# completion keeps data availability smooth for the pipeline
GROUPS = (2048,) * 12
CHUNK = 128                  # rows per layer-4 matmul (stationary M)
N_CHUNKS = RPC // CHUNK      # 196
# pooling flush boundaries (chunk counts)
FLUSH_AT = (49, 98, 147, 196)
SLOTS = 49                   # max slots per pooling batch (PSUM tile)
N_BATCH = len(FLUSH_AT)
ACT_COLS = 576               # ReLU cols on ACT; rest on DVE
ACT_COLS_FLUSH = 576
N_WARM = 6                   # warmup matmuls on the constants

F32 = mybir.dt.float32
BF16 = mybir.dt.bfloat16
FP8 = mybir.dt.float8e4
AF = mybir.ActivationFunctionType
ALU = mybir.AluOpType


def build_bass(has_bias=False):
    nc = bacc.Bacc()

    CW_COLS = (OUT + 1) if has_bias else OUT
    cw_ext = nc.declare_dram_parameter("cw", [F, CW_COLS], BF16, isOutput=False)
    xt_ext = nc.declare_dram_parameter("xt", [F, F + RPC], FP8, isOutput=False)
    out_ext = nc.declare_dram_parameter(
        "out", [F, 2, N_BATCH, OUT], BF16, isOutput=True
    )

    with tile.TileContext(nc) as tc:
        with (
            tc.tile_pool(name="stat", bufs=1) as stat,
            tc.tile_pool(name="hp", bufs=4) as hp,
            tc.tile_pool(name="ps_h", bufs=3, space="PSUM") as ps_h,
            tc.tile_pool(name="ps_p", bufs=2, space="PSUM") as ps_p,
        ):
            cw = stat.tile([F, CW_COLS], BF16)
            xsb = stat.tile([F, F + RPC], FP8)
            parts = stat.tile([F, 2, N_BATCH, OUT], BF16)
            e_b = stat.tile([F, OUT, SLOTS], BF16)
            pe_b = stat.tile([F, OUT, SLOTS], BF16)

            # bf16 constants ride the ACT engine's HWDGE queue, ahead of
            # its table load, so they land before the Sync queue warms up
            nc.scalar.dma_start(out=cw[:], in_=cw_ext[:])
            w4at = cw[:, :OUT]
            # x streaming: wct alone first (so ldweights fires as early
            # as possible), the small first block, then ascending groups
            nc.sync.dma_start(out=xsb[:, :F], in_=xt_ext[:, :F])
            wct = xsb[:, :F]
            nc.tensor.ldweights(wct)  # PE observes the first DMA early
            nc.sync.dma_start(out=xsb[:, F : F + T0], in_=xt_ext[:, F : F + T0])
            c0 = F + T0
            for g in GROUPS:
                nc.sync.dma_start(out=xsb[:, c0 : c0 + g], in_=xt_ext[:, c0 : c0 + g])
                c0 += g

            # warmup matmuls on the wct block: keep the PE p-state ramp
            # going while the first x block is still in flight
            warm = ps_h.tile([F, TILE], F32, tag="h3p")
            for _ in range(N_WARM):
                nc.tensor.matmul(
                    warm[:, :F], wct, xsb[:, :F], start=True, stop=True
                )

            bc = None
            if has_bias:
                bc = stat.tile([F, 1], F32)
                nc.vector.tensor_copy(bc[:], cw[:, OUT : OUT + 1])

            state = {"chunk": 0, "pp": None, "base": 0, "bi": 0, "pend": None}

            def flush_batch():
                # record the completed batch; its ops are emitted after the
                # NEXT tile's relu so relu stays ahead in the DVE queue
                state["pend"] = (state["bi"], state["chunk"] - state["base"],
                                 state["pp"])
                state["bi"] += 1
                state["base"] = state["chunk"]

            def emit_pending():
                if state["pend"] is None:
                    return
                bi, n_slots, pp = state["pend"]
                state["pend"] = None
                sl = slice(0, n_slots)
                nc.scalar.activation(e_b[:, :, sl], pp[:, :, sl], AF.Exp)
                nc.vector.tensor_tensor(
                    pe_b[:, :, sl], pp[:, :, sl], e_b[:, :, sl], ALU.mult
                )
                with nc.allow_low_precision("partials rounded once to bf16"):
                    nc.vector.tensor_reduce(
                        parts[:, 0, bi, :], e_b[:, :, sl],
                        mybir.AxisListType.X, ALU.add,
                    )
                    nc.vector.tensor_reduce(
                        parts[:, 1, bi, :], pe_b[:, :, sl],
                        mybir.AxisListType.X, ALU.add,
                    )

            def do_tile(x0, rows, no_act=False):
                n_ch = rows // CHUNK
                # on tiles that emit a pooling flush, shift relu columns
                # toward ACT so the DVE has headroom for the flush ops
                a_cols = ACT_COLS_FLUSH if state["pend"] is not None else ACT_COLS
                a_cols = 0 if no_act else min(a_cols, rows)
                h3p = ps_h.tile([F, TILE], F32, tag="h3p")
                # one matmul per 512-col PSUM bank (f32 free-dim limit)
                for c in range(0, rows, 512):
                    cw_ = min(512, rows - c)
                    nc.tensor.matmul(
                        h3p[:, c : c + cw_], wct, xsb[:, x0 + c : x0 + c + cw_],
                        start=True, stop=True,
                    )
                hsb = hp.tile([F, TILE], BF16, tag="hsb")
                if a_cols:
                    if has_bias:
                        nc.scalar.activation(
                            hsb[:, :a_cols], h3p[:, :a_cols], AF.Relu,
                            bias=bc[:], scale=1.0,
                        )
                    else:
                        nc.scalar.activation(
                            hsb[:, :a_cols], h3p[:, :a_cols], AF.Relu
                        )
                if a_cols < rows:
                    if has_bias:
                        nc.vector.tensor_scalar(
                            hsb[:, a_cols:rows], h3p[:, a_cols:rows],
                            bc[:], 0.0, ALU.add, ALU.max,
                        )
                    else:
                        nc.vector.tensor_scalar_max(
                            hsb[:, a_cols:rows], h3p[:, a_cols:rows], 0.0
                        )
                emit_pending()
                # DVE-half chunks first: the DVE relu lands earlier than
                # ACT's, so the PE resumes layer 4 sooner (slot order is
                # irrelevant -- the pooling sums over all rows)
                a_ch = a_cols // CHUNK
                order = list(range(a_ch, n_ch)) + list(range(a_ch))
                for j in order:
                    c = state["chunk"]
                    s = c - state["base"]
                    if s == 0:
                        state["pp"] = ps_p.tile(
                            [F, OUT, SLOTS], F32, tag="pp", name="pp"
                        )
                    nc.tensor.matmul(
                        state["pp"][:, :, s],
                        hsb[:, j * CHUNK : (j + 1) * CHUNK], w4at,
                        start=True, stop=True,
                    )
                    state["chunk"] = c + 1
                    if state["chunk"] == FLUSH_AT[state["bi"]]:
                        flush_batch()

            # first tile: DVE-only relu so nothing waits on the ACT
            # table load; it streams in parallel with the x DMAs
            do_tile(F, T0, no_act=True)
            for t in range(N_TILES):
                do_tile(F + T0 + t * TILE, TILE)
            emit_pending()

            nc.sync.dma_start(out=out_ext[:], in_=parts[:])

    nc.finalize()
    return nc


_CACHED = {}
TRACE = False
LAST = {}


def kernel(x, W1, b1, W2, b2, W3, b3, W4, b4, alpha):
    f64 = np.float64
    x2 = np.asarray(x, np.float32).reshape(N_ROWS, F)
    W1, b1, W2, b2, W3, b3, W4, b4, alpha = [
        np.asarray(a, f64) for a in (W1, b1, W2, b2, W3, b3, W4, b4, alpha)
    ]

    # fold the linear layers (exact in f64)
    Wc = W3 @ W2 @ W1
    bc = W3 @ (W2 @ b1 + b2) + b3
    alpha_safe = np.where(np.abs(alpha) < 1e-12, 1e-12, alpha)
    W4a = alpha_safe[:, None] * W4

    # pad rows to 8*25088 with zeros; contribution removed on the host
    n_pad = ROWS_PAD - N_ROWS
    xp = np.concatenate([x2, np.zeros((n_pad, F), np.float32)], axis=0)
    xT = np.ascontiguousarray(xp.T).astype(ml_dtypes.float8_e4m3)

    has_bias = bool(np.any(bc != 0.0))
    key = ("nc", has_bias)
    if key not in _CACHED:
        _CACHED[key] = build_bass(has_bias)
    nc = _CACHED[key]

    wct_np = np.ascontiguousarray(Wc.T).astype(ml_dtypes.float8_e4m3)
    w4at_np = np.ascontiguousarray(W4a.T).astype(ml_dtypes.bfloat16)
    parts_list = [w4at_np]
    if has_bias:
        parts_list.append(
            bc.reshape(F, 1).astype(np.float32).astype(ml_dtypes.bfloat16)
        )
    cw_np = np.ascontiguousarray(np.concatenate(parts_list, axis=1))

    in_maps = []
    for c in range(N_CORES):
        shard = xT[:, c * RPC : (c + 1) * RPC]
        in_maps.append({
            "cw": cw_np,
            "xt": np.ascontiguousarray(np.concatenate([wct_np, shard], axis=1)),
        })

    res = run_bass_kernel_spmd(
        nc, in_maps, core_ids=list(range(N_CORES)), trace=TRACE
    )
    LAST["res"] = res
    outs = np.stack(
        [np.asarray(r["out"], f64) for r in res.results]
    )  # [8, F, 2, NB, OUT]
    den = outs[:, :, 0].sum(axis=(0, 1, 2))  # [5]
    num = outs[:, :, 1].sum(axis=(0, 1, 2))  # [5]

    # remove the zero-pad rows' contribution (each pad row: h0 = relu(bc))
    h0 = np.maximum(bc, 0.0)
    pt0 = W4a @ h0
    den -= n_pad * np.exp(pt0)
    num -= n_pad * pt0 * np.exp(pt0)

    out = num / (alpha_safe * den) + b4
    return out[None, :].astype(np.float32)


# revision 14
# speedup vs baseline: 1.0263x; 1.0263x over previous
"""Trainium2 Bass kernel for nn_AdaptivePoolingClassifier (8 NeuronCores).

Math: the reference MLP is linear up to its single ReLU, so W1..W3 fold
into one 128x128 matrix on the host:
    h   = relu(x @ Wc^T + bc)       Wc = W3 W2 W1 ; bc = W3(W2 b1+b2)+b3
    p   = h @ W4^T + b4
    out = sum_n p * softmax(alpha*p, axis=1)

Device computes (rows sharded 8 ways):
    pt  = h @ (diag(alpha) W4)^T        # = alpha*(p - b4), [rows, 5]
    den_partial = sum_rows exp(pt) ; num_partial = sum_rows pt*exp(pt)
Host finishes: out_o = num_o/(alpha_o*den_o) + b4_o.

Dtypes: layer 1 runs all-fp8 (x and Wc — x noise averages out over the
200k-row softmax pool, Wc costs ~5e-3 systematic, well under the gate);
layer 4 runs all-bf16 (h, W4a — fp8 W4a was the error killer at 3e-2,
and mixed-dtype matmuls lower to a slow PE path so operands must match).

Layout: x is transposed on the host to [128(feat), rows] so features sit
on SBUF partitions for the folded matmul; layer-4 uses h chunks as the
matmul *stationary* operand so pt lands rows-on-partitions, making the
pooling full-width [128, 5*slots] ops instead of lane-starved [5, n]
ones. The whole fp8 x shard (25KB/partition) stays resident in SBUF via
5 ascending-size DMAs. ReLU is split between ACT and DVE by column range
(chunk-aligned). A few warmup matmuls on the constants keep the PE
p-state ramp going while the first x block streams in.
"""

import numpy as np
import ml_dtypes

from concourse import bacc, mybir, tile
from concourse.bass_utils import run_bass_kernel_spmd

N_CORES = 8
N_ROWS = 200000
F = 128
OUT = 5

ROWS_PAD = 200704            # 8 * 25088
RPC = ROWS_PAD // N_CORES    # rows per core = 25088 = 512 + 24*1024
T0 = 512                     # first tile
TILE = 1024                  # steady-state compute tile (2 PSUM banks)
N_TILES = (RPC - T0) // TILE  # 24
# uniform DMA group size after the wct and T0 transfers: fine-grained
# completion keeps data availability smooth for the pipeline
GROUPS = (2048,) * 12
CHUNK = 128                  # rows per layer-4 matmul (stationary M)
N_CHUNKS = RPC // CHUNK      # 196
# pooling flush boundaries (chunk counts)
FLUSH_AT = (49, 98, 147, 196)
SLOTS = 49                   # max slots per pooling batch (PSUM tile)
N_BATCH = len(FLUSH_AT)
ACT_COLS = 576               # ReLU cols on ACT; rest on DVE
ACT_COLS_FLUSH = 576
N_WARM = 6                   # warmup matmuls on the constants

F32 = mybir.dt.float32
BF16 = mybir.dt.bfloat16
FP8 = mybir.dt.float8e4
AF = mybir.ActivationFunctionType
ALU = mybir.AluOpType


def build_bass(has_bias=False):
    nc = bacc.Bacc()

    CW_COLS = (OUT + 1) if has_bias else OUT
    cw_ext = nc.declare_dram_parameter("cw", [F, CW_COLS], BF16, isOutput=False)
    xt_ext = nc.declare_dram_parameter("xt", [F, F + RPC], FP8, isOutput=False)
    out_ext = nc.declare_dram_parameter(
        "out", [F, 2, N_BATCH, OUT], BF16, isOutput=True
    )

    with tile.TileContext(nc) as tc:
        with (
            tc.tile_pool(name="stat", bufs=1) as stat,
            tc.tile_pool(name="hp", bufs=4) as hp,
            tc.tile_pool(name="ps_h", bufs=3, space="PSUM") as ps_h,
            tc.tile_pool(name="ps_p", bufs=2, space="PSUM") as ps_p,
        ):
            cw = stat.tile([F, CW_COLS], BF16)
            xsb = stat.tile([F, F + RPC], FP8)
            parts = stat.tile([F, 2, N_BATCH, OUT], BF16)
            e_b = stat.tile([F, OUT, SLOTS], BF16)
            pe_b = stat.tile([F, OUT, SLOTS], BF16)

            # bf16 constants ride the ACT engine's HWDGE queue, ahead of
            # its table load, so they land before the Sync queue warms up
            nc.scalar.dma_start(out=cw[:], in_=cw_ext[:])
            w4at = cw[:, :OUT]
            # x streaming: wct alone first (so ldweights fires as early
            # as possible), the small first block, then ascending groups
            nc.sync.dma_start(out=xsb[:, :F], in_=xt_ext[:, :F])
            wct = xsb[:, :F]
            nc.tensor.ldweights(wct)  # PE observes the first DMA early
            nc.sync.dma_start(out=xsb[:, F : F + T0], in_=xt_ext[:, F : F + T0])
            c0 = F + T0
            for g in GROUPS:
                nc.sync.dma_start(out=xsb[:, c0 : c0 + g], in_=xt_ext[:, c0 : c0 + g])
                c0 += g

            # warmup matmuls on the wct block: keep the PE p-state ramp
            # going while the first x block is still in flight
            warm = ps_h.tile([F, TILE], F32, tag="h3p")
            for _ in range(N_WARM):
                nc.tensor.matmul(
                    warm[:, :F], wct, xsb[:, :F], start=True, stop=True
                )

            bc = None
            if has_bias:
                bc = stat.tile([F, 1], F32)
                nc.vector.tensor_copy(bc[:], cw[:, OUT : OUT + 1])

            state = {"chunk": 0, "pp": None, "base": 0, "bi": 0, "pend": None}

            def flush_batch():
                # record the completed batch; its ops are emitted after the
                # NEXT tile's relu so relu stays ahead in the DVE queue
                state["pend"] = (state["bi"], state["chunk"] - state["base"],
                                 state["pp"])
                state["bi"] += 1
                state["base"] = state["chunk"]

            def emit_pending():
                if state["pend"] is None:
                    return
                bi, n_slots, pp = state["pend"]
                state["pend"] = None
                sl = slice(0, n_slots)
                nc.scalar.activation(e_b[:, :, sl], pp[:, :, sl], AF.Exp)
                nc.vector.tensor_tensor(
                    pe_b[:, :, sl], pp[:, :, sl], e_b[:, :, sl], ALU.mult
                )
                with nc.allow_low_precision("partials rounded once to bf16"):
                    nc.vector.tensor_reduce(
                        parts[:, 0, bi, :], e_b[:, :, sl],
                        mybir.AxisListType.X, ALU.add,
                    )
                    nc.vector.tensor_reduce(
                        parts[:, 1, bi, :], pe_b[:, :, sl],
                        mybir.AxisListType.X, ALU.add,
                    )

            def do_tile(x0, rows, no_act=False):
                n_ch = rows // CHUNK
                # on tiles that emit a pooling flush, shift relu columns
                # toward ACT so the DVE has headroom for the flush ops
                a_cols = ACT_COLS_FLUSH if state["pend"] is not None else ACT_COLS
                a_cols = 0 if no_act else min(a_cols, rows)
                h3p = ps_h.tile([F, TILE], F32, tag="h3p")
                # one matmul per 512-col PSUM bank (f32 free-dim limit)
                for c in range(0, rows, 512):
                    cw_ = min(512, rows - c)
                    nc.tensor.matmul(
                        h3p[:, c : c + cw_], wct, xsb[:, x0 + c : x0 + c + cw_],
                        start=True, stop=True,
                    )
                hsb = hp.tile([F, TILE], BF16, tag="hsb")
                if a_cols:
                    if has_bias:
                        nc.scalar.activation(
                            hsb[:, :a_cols], h3p[:, :a_cols], AF.Relu,
                            bias=bc[:], scale=1.0,
                        )
                    else:
                        nc.scalar.activation(
                            hsb[:, :a_cols], h3p[:, :a_cols], AF.Relu
                        )
                if a_cols < rows:
                    if has_bias:
                        nc.vector.tensor_scalar(
                            hsb[:, a_cols:rows], h3p[:, a_cols:rows],
                            bc[:], 0.0, ALU.add, ALU.max,
                        )
                    else:
                        nc.vector.tensor_scalar_max(
                            hsb[:, a_cols:rows], h3p[:, a_cols:rows], 0.0
                        )
                emit_pending()
                for j in range(n_ch):
                    c = state["chunk"]
                    s = c - state["base"]
                    if s == 0:
                        state["pp"] = ps_p.tile(
                            [F, OUT, SLOTS], F32, tag="pp", name="pp"
                        )
                    nc.tensor.matmul(
                        state["pp"][:, :, s],
                        hsb[:, j * CHUNK : (j + 1) * CHUNK], w4at,
                        start=True, stop=True,
                    )
                    state["chunk"] = c + 1
                    if state["chunk"] == FLUSH_AT[state["bi"]]:
                        flush_batch()

            # first tile: DVE-only relu so nothing waits on the ACT
            # table load; it streams in parallel with the x DMAs
            do_tile(F, T0, no_act=True)
            for t in range(N_TILES):
                do_tile(F + T0 + t * TILE, TILE)
            emit_pending()

            nc.sync.dma_start(out=out_ext[:], in_=parts[:])

    nc.finalize()
    return nc


_CACHED = {}
TRACE = False
LAST = {}


def kernel(x, W1, b1, W2, b2, W3, b3, W4, b4, alpha):
    f64 = np.float64
    x2 = np.asarray(x, np.float32).reshape(N_ROWS, F)
    W1, b1, W2, b2, W3, b3, W4, b4, alpha = [
        np.asarray(a, f64) for a in (W1, b1, W2, b2, W3, b3, W4, b4, alpha)
    ]

    # fold the linear layers (exact in f64)
    Wc = W3 @ W2 @ W1
    bc = W3 @ (W2 @ b1 + b2) + b3
    alpha_safe = np.where(np.abs(alpha) < 1e-12, 1e-12, alpha)
    W4a = alpha_safe[:, None] * W4

    # pad rows to 8*25088 with zeros; contribution removed on the host
    n_pad = ROWS_PAD - N_ROWS
    xp = np.concatenate([x2, np.zeros((n_pad, F), np.float32)], axis=0)
    xT = np.ascontiguousarray(xp.T).astype(ml_dtypes.float8_e4m3)

    has_bias = bool(np.any(bc != 0.0))
    key = ("nc", has_bias)
    if key not in _CACHED:
        _CACHED[key] = build_bass(has_bias)
    nc = _CACHED[key]

    wct_np = np.ascontiguousarray(Wc.T).astype(ml_dtypes.float8_e4m3)
    w4at_np = np.ascontiguousarray(W4a.T).astype(ml_dtypes.bfloat16)
    parts_list = [w4at_np]
    if has_bias:
        parts_list.append(
            bc.reshape(F, 1).astype(np.float32).astype(ml_dtypes.bfloat16)
        )
    cw_np = np.ascontiguousarray(np.concatenate(parts_list, axis=1))

    in_maps = []
    for c in range(N_CORES):
        shard = xT[:, c * RPC : (c + 1) * RPC]
        in_maps.append({
            "cw": cw_np,
            "xt": np.ascontiguousarray(np.concatenate([wct_np, shard], axis=1)),
        })

    res = run_bass_kernel_spmd(
        nc, in_maps, core_ids=list(range(N_CORES)), trace=TRACE
    )
    LAST["res"] = res
    outs = np.stack(
        [np.asarray(r["out"], f64) for r in res.results]
    )  # [8, F, 2, NB, OUT]
    den = outs[:, :, 0].sum(axis=(0, 1, 2))  # [5]
    num = outs[:, :, 1].sum(axis=(0, 1, 2))  # [5]

    # remove the zero-pad rows' contribution (each pad row: h0 = relu(bc))
    h0 = np.maximum(bc, 0.0)
    pt0 = W4a @ h0
    den -= n_pad * np.exp(pt0)
    num -= n_pad * pt0 * np.exp(pt0)

    out = num / (alpha_safe * den) + b4
    return out[None, :].astype(np.float32)


# revision 16
# speedup vs baseline: 1.0286x; 1.0023x over previous
"""Trainium2 Bass kernel for nn_AdaptivePoolingClassifier (8 NeuronCores).

Math: the reference MLP is linear up to its single ReLU, so W1..W3 fold
into one 128x128 matrix on the host:
    h   = relu(x @ Wc^T + bc)       Wc = W3 W2 W1 ; bc = W3(W2 b1+b2)+b3
    p   = h @ W4^T + b4
    out = sum_n p * softmax(alpha*p, axis=1)

Device computes (rows sharded 8 ways):
    pt  = h @ (diag(alpha) W4)^T        # = alpha*(p - b4), [rows, 5]
    den_partial = sum_rows exp(pt) ; num_partial = sum_rows pt*exp(pt)
Host finishes: out_o = num_o/(alpha_o*den_o) + b4_o.

Dtypes: layer 1 runs all-fp8 (x and Wc — x noise averages out over the
200k-row softmax pool, Wc costs ~5e-3 systematic, well under the gate);
layer 4 runs all-bf16 (h, W4a — fp8 W4a was the error killer at 3e-2,
and mixed-dtype matmuls lower to a slow PE path so operands must match).

Layout: x is transposed on the host to [128(feat), rows] so features sit
on SBUF partitions for the folded matmul; layer-4 uses h chunks as the
matmul *stationary* operand so pt lands rows-on-partitions, making the
pooling full-width [128, 5*slots] ops instead of lane-starved [5, n]
ones. The whole fp8 x shard (25KB/partition) stays resident in SBUF via
5 ascending-size DMAs. ReLU is split between ACT and DVE by column range
(chunk-aligned). A few warmup matmuls on the constants keep the PE
p-state ramp going while the first x block streams in.
"""

import numpy as np
import ml_dtypes

from concourse import bacc, mybir, tile
from concourse.bass_utils import run_bass_kernel_spmd

N_CORES = 8
N_ROWS = 200000
F = 128
OUT = 5

ROWS_PAD = 200704            # 8 * 25088
RPC = ROWS_PAD // N_CORES    # rows per core = 25088 = 512 + 24*1024
T0 = 512                     # first tile
TILE = 1024                  # steady-state compute tile (2 PSUM banks)
N_TILES = (RPC - T0) // TILE  # 24
# uniform DMA group size after the wct and T0 transfers: fine-grained
# completion keeps data availability smooth for the pipeline
GROUPS = (2048,) * 12
CHUNK = 128                  # rows per layer-4 matmul (stationary M)
N_CHUNKS = RPC // CHUNK      # 196
# pooling flush boundaries (chunk counts)
FLUSH_AT = (49, 98, 147, 196)
SLOTS = 49                   # max slots per pooling batch (PSUM tile)
N_BATCH = len(FLUSH_AT)
ACT_COLS = 576               # ReLU cols on ACT; rest on DVE
ACT_COLS_FLUSH = 576
N_WARM = 24                  # warmup matmuls on garbage (no data deps)

F32 = mybir.dt.float32
BF16 = mybir.dt.bfloat16
FP8 = mybir.dt.float8e4
AF = mybir.ActivationFunctionType
ALU = mybir.AluOpType


def build_bass(has_bias=False):
    nc = bacc.Bacc()

    CW_COLS = (OUT + 1) if has_bias else OUT
    cw_ext = nc.declare_dram_parameter("cw", [F, CW_COLS], BF16, isOutput=False)
    xt_ext = nc.declare_dram_parameter("xt", [F, F + RPC], FP8, isOutput=False)
    out_ext = nc.declare_dram_parameter(
        "out", [F, 2, N_BATCH, OUT], BF16, isOutput=True
    )

    with tile.TileContext(nc) as tc:
        with (
            tc.tile_pool(name="stat", bufs=1) as stat,
            tc.tile_pool(name="hp", bufs=4) as hp,
            tc.tile_pool(name="ps_h", bufs=3, space="PSUM") as ps_h,
            tc.tile_pool(name="ps_p", bufs=2, space="PSUM") as ps_p,
        ):
            cw = stat.tile([F, CW_COLS], BF16)
            xsb = stat.tile([F, F + RPC], FP8)
            parts = stat.tile([F, 2, N_BATCH, OUT], BF16)
            # exp and pt*exp share one tile so ONE reduce covers both
            epe = stat.tile([F, 2, OUT, SLOTS], BF16)
            # zeroed at boot by the otherwise-idle GpSimd: warmup operands
            wsrc = stat.tile([F, 3 * F], FP8)
            nc.gpsimd.memset(wsrc[:], 0.0)

            # bf16 constants ride the ACT engine's HWDGE queue, ahead of
            # its table load, so they land before the Sync queue warms up
            nc.scalar.dma_start(out=cw[:], in_=cw_ext[:])
            w4at = cw[:, :OUT]
            # x streaming: wct alone first, the small first block, then
            # uniform groups
            nc.sync.dma_start(out=xsb[:, :F], in_=xt_ext[:, :F])
            wct = xsb[:, :F]
            nc.sync.dma_start(out=xsb[:, F : F + T0], in_=xt_ext[:, F : F + T0])
            c0 = F + T0
            for g in GROUPS:
                nc.sync.dma_start(out=xsb[:, c0 : c0 + g], in_=xt_ext[:, c0 : c0 + g])
                c0 += g

            # pre-data warmup: matmuls on garbage SBUF have no deps, so
            # the PE p-state ramp starts at engine boot instead of first
            # data arrival; contents are discarded (start=True resets)
            warm = ps_h.tile([F, TILE], F32, tag="h3p")
            for _ in range(N_WARM):
                nc.tensor.matmul(
                    warm[:, :2 * F], wsrc[:, :F], wsrc[:, F : 3 * F],
                    start=True, stop=True,
                )
            nc.tensor.ldweights(wct)  # prefetch wct behind the warmups

            bc = None
            if has_bias:
                bc = stat.tile([F, 1], F32)
                nc.vector.tensor_copy(bc[:], cw[:, OUT : OUT + 1])

            state = {"chunk": 0, "pp": None, "base": 0, "bi": 0, "pend": None}

            def flush_batch():
                # record the completed batch; its ops are emitted after the
                # NEXT tile's relu so relu stays ahead in the DVE queue
                state["pend"] = (state["bi"], state["chunk"] - state["base"],
                                 state["pp"])
                state["bi"] += 1
                state["base"] = state["chunk"]

            def emit_pending():
                if state["pend"] is None:
                    return
                bi, n_slots, pp = state["pend"]
                state["pend"] = None
                sl = slice(0, n_slots)
                nc.scalar.activation(epe[:, 0, :, sl], pp[:, :, sl], AF.Exp)
                nc.vector.tensor_tensor(
                    epe[:, 1, :, sl], pp[:, :, sl], epe[:, 0, :, sl], ALU.mult
                )
                with nc.allow_low_precision("partials rounded once to bf16"):
                    nc.vector.tensor_reduce(
                        parts[:, :, bi, :], epe[:, :, :, sl],
                        mybir.AxisListType.X, ALU.add,
                    )

            def do_tile(x0, rows, no_act=False):
                n_ch = rows // CHUNK
                # on tiles that emit a pooling flush, shift relu columns
                # toward ACT so the DVE has headroom for the flush ops
                a_cols = ACT_COLS_FLUSH if state["pend"] is not None else ACT_COLS
                a_cols = 0 if no_act else min(a_cols, rows)
                h3p = ps_h.tile([F, TILE], F32, tag="h3p")
                # one matmul per 512-col PSUM bank (f32 free-dim limit)
                for c in range(0, rows, 512):
                    cw_ = min(512, rows - c)
                    nc.tensor.matmul(
                        h3p[:, c : c + cw_], wct, xsb[:, x0 + c : x0 + c + cw_],
                        start=True, stop=True,
                    )
                hsb = hp.tile([F, TILE], BF16, tag="hsb")
                if a_cols:
                    if has_bias:
                        nc.scalar.activation(
                            hsb[:, :a_cols], h3p[:, :a_cols], AF.Relu,
                            bias=bc[:], scale=1.0,
                        )
                    else:
                        nc.scalar.activation(
                            hsb[:, :a_cols], h3p[:, :a_cols], AF.Relu
                        )
                if a_cols < rows:
                    if has_bias:
                        nc.vector.tensor_scalar(
                            hsb[:, a_cols:rows], h3p[:, a_cols:rows],
                            bc[:], 0.0, ALU.add, ALU.max,
                        )
                    else:
                        nc.vector.tensor_scalar_max(
                            hsb[:, a_cols:rows], h3p[:, a_cols:rows], 0.0
                        )
                emit_pending()
                for j in range(n_ch):
                    c = state["chunk"]
                    s = c - state["base"]
                    if s == 0:
                        state["pp"] = ps_p.tile(
                            [F, OUT, SLOTS], F32, tag="pp", name="pp"
                        )
                    nc.tensor.matmul(
                        state["pp"][:, :, s],
                        hsb[:, j * CHUNK : (j + 1) * CHUNK], w4at,
                        start=True, stop=True,
                    )
                    state["chunk"] = c + 1
                    if state["chunk"] == FLUSH_AT[state["bi"]]:
                        flush_batch()

            # first tile: DVE-only relu so nothing waits on the ACT
            # table load; it streams in parallel with the x DMAs
            do_tile(F, T0, no_act=True)
            for t in range(N_TILES):
                do_tile(F + T0 + t * TILE, TILE)
            emit_pending()

            nc.sync.dma_start(out=out_ext[:], in_=parts[:])

    nc.finalize()
    return nc


_CACHED = {}
TRACE = False
LAST = {}


def kernel(x, W1, b1, W2, b2, W3, b3, W4, b4, alpha):
    f64 = np.float64
    x2 = np.asarray(x, np.float32).reshape(N_ROWS, F)
    W1, b1, W2, b2, W3, b3, W4, b4, alpha = [
        np.asarray(a, f64) for a in (W1, b1, W2, b2, W3, b3, W4, b4, alpha)
    ]

    # fold the linear layers (exact in f64)
    Wc = W3 @ W2 @ W1
    bc = W3 @ (W2 @ b1 + b2) + b3
    alpha_safe = np.where(np.abs(alpha) < 1e-12, 1e-12, alpha)
    W4a = alpha_safe[:, None] * W4

    # pad rows to 8*25088 with zeros; contribution removed on the host
    n_pad = ROWS_PAD - N_ROWS
    xp = np.concatenate([x2, np.zeros((n_pad, F), np.float32)], axis=0)
    xT = np.ascontiguousarray(xp.T).astype(ml_dtypes.float8_e4m3)

    has_bias = bool(np.any(bc != 0.0))
    key = ("nc", has_bias)
    if key not in _CACHED:
        _CACHED[key] = build_bass(has_bias)
    nc = _CACHED[key]

    wct_np = np.ascontiguousarray(Wc.T).astype(ml_dtypes.float8_e4m3)
    w4at_np = np.ascontiguousarray(W4a.T).astype(ml_dtypes.bfloat16)
    parts_list = [w4at_np]
    if has_bias:
        parts_list.append(
            bc.reshape(F, 1).astype(np.float32).astype(ml_dtypes.bfloat16)
        )
    cw_np = np.ascontiguousarray(np.concatenate(parts_list, axis=1))

    in_maps = []
    for c in range(N_CORES):
        shard = xT[:, c * RPC : (c + 1) * RPC]
        in_maps.append({
            "cw": cw_np,
            "xt": np.ascontiguousarray(np.concatenate([wct_np, shard], axis=1)),
        })

    res = run_bass_kernel_spmd(
        nc, in_maps, core_ids=list(range(N_CORES)), trace=TRACE
    )
    LAST["res"] = res
    outs = np.stack(
        [np.asarray(r["out"], f64) for r in res.results]
    )  # [8, F, 2, NB, OUT]
    den = outs[:, :, 0].sum(axis=(0, 1, 2))  # [5]
    num = outs[:, :, 1].sum(axis=(0, 1, 2))  # [5]

    # remove the zero-pad rows' contribution (each pad row: h0 = relu(bc))
    h0 = np.maximum(bc, 0.0)
    pt0 = W4a @ h0
    den -= n_pad * np.exp(pt0)
    num -= n_pad * pt0 * np.exp(pt0)

    out = num / (alpha_safe * den) + b4
    return out[None, :].astype(np.float32)
